# revision 26
# baseline (speedup 1.0000x reference)
"""Trainium2 Bass kernel for nn_DCNConvModule (modulated deformable conv
+ GroupNorm(1) + ReLU).

Sharding: 8 cores; core (2b + h) computes sample b, output rows [32h, 32h+32).
GroupNorm statistics are per-sample -> tiny AllReduce of (sum, sumsq) within
core pairs [[0,1],[2,3],[4,5],[6,7]].

Per-core algorithm (pixel-major "px" = 2048 output pixels on 16 tiles of 128),
fp16 data path, software-pipelined per half (1024 px):
  1. offset conv as 9-tap PE matmuls (fp16) -> off [27, px-half] -> PE
     transpose -> [px, 27]; coef/index math on DVE in fp32.
  2. pair-row gather table in DRAM: entry (y, x) = [ch of (y,x) | ch of
     (y+1,x)] fp16, 66x66 grid (+1 coordinate shift; zero pad cols/rows make
     bilinear corner validity masking implicit).  One dma_gather index
     fetches entries (e, e+1) = all 4 bilinear corners (2KB).
  3. per (tap, px-tile): corner combine + transpose fused into PE matmuls
     with diagonal coefficient matrices: valT[c,px] = sum_corner
     g_corner^T @ diag(coef_corner); 4 corners accumulate in PSUM fp32.
     diag matrices built by DVE tensor_scalar (fp16 identity x per-px coef).
  4. valT (fp16) x w2 (fp16) accumulate y[256, px] in PSUM across taps
     (512-px-wide matmuls, one PSUM bank per (oh, quad)).
  5. GN: per-partition sums via ACT accumulators, ones-matmul partition
     reduce, pair AllReduce, normalize+ReLU as one ACT op per chunk.
The half-1 prefix (offset conv / coefs / indices) is emitted between half-0
main-loop tap blocks so it executes under the gather DMA shadow.
"""
import contextlib
import numpy as np

K = 3
KK = 9
C = 256
CO = 256
H = 64
W = 64
B = 4
GW = 66                  # padded grid width
NENT = GW * GW           # 4356 pair-row entries (y 0..65, x 0..65)
TBL_ROWS = NENT + 4      # slack entries (zeros)
NWIN = TBL_ROWS - 2      # overlapping 1024-elem windows, stride 512
PXT = 16                 # 128-pixel tiles per core
NPX = PXT * 128          # 2048 pixels per core
GN_EPS = 1e-5
NCORES = 8

_cache = {}


# ----------------------------------------------------------------- host prep
def prep_per_core(x, w_off, b_off, w, b, gamma, beta):
    """Build the 8 per-core input maps (all numpy, layout-only work)."""
    ki = np.arange(KK) // K
    kj = np.arange(KK) % K

    # conv_offset lhsT  [128, 2, 9, 27]: [c', cc, tap, o]
    woff_r = np.ascontiguousarray(
        w_off.reshape(27, 2, 128, K, K).transpose(2, 1, 3, 4, 0)
        .reshape(128, 2, 9, 27)).astype(np.float16)
    # main DCN lhsT  [128, 9, 2, 2, 128]: [c', k, cc, oh, o']
    w2 = np.ascontiguousarray(
        w.reshape(2, 128, 2, 128, KK).transpose(3, 4, 2, 0, 1)
    ).astype(np.float16)
    boff = b_off.reshape(27, 1).astype(np.float32).copy()
    bvec = np.ascontiguousarray(b.reshape(2, 128).T).astype(np.float32)
    gam2 = np.ascontiguousarray(gamma.reshape(2, 128).T).astype(np.float32)
    bet2 = np.ascontiguousarray(beta.reshape(2, 128).T).astype(np.float32)

    p = np.arange(128)
    basex = (p[:, None] % 64 + kj[None, :]).astype(np.float32)  # [128, 9]

    x_pad = np.zeros((B, C, H + 2, W + 2), np.float32)
    x_pad[:, :, 1:H+1, 1:W+1] = x

    rep = np.zeros((16, 128), np.float32)
    rep[np.arange(128) % 16, np.arange(128)] = 1.0

    in_maps = []
    tbl_cache = {}
    for core in range(NCORES):
        bi, hh = core // 2, core % 2
        h0 = hh * 32
        if bi not in tbl_cache:
            # pair-row entries: ent[y, x] = [ch of (y,x) | ch of (y+1,x)]
            gridt = np.zeros((GW + 1, GW, C), np.float16)
            gridt[:GW] = x_pad[bi].transpose(1, 2, 0).astype(np.float16)
            ent = np.concatenate([gridt[:GW], gridt[1:GW+1]], axis=2)
            tbl = np.zeros((TBL_ROWS, 2 * C), np.float16)
            tbl[:NENT] = ent.reshape(NENT, 2 * C)
            tbl_cache[bi] = tbl
        slab = x_pad[bi][:, h0:h0+34, :]                    # [256, 34, 66]
        xc = np.ascontiguousarray(
            slab.reshape(2, 128, 34 * 66).transpose(1, 0, 2)).astype(np.float16)
        t = np.arange(PXT)
        basey = np.ascontiguousarray(
            (h0 + (t[None, :, None] * 128 + p[:, None, None]) // 64
             + ki[None, None, :])).astype(np.float32)
        in_maps.append(dict(
            xt=tbl_cache[bi], xc=xc,
            woff=woff_r, boff=boff, w2=w2,
            bvec=bvec, gam2=gam2, bet2=bet2,
            basey=basey, basex=basex, rep16=rep,
        ))
    return in_maps


# --------------------------------------------------------------- bass kernel
def build_module(use_collective=True):
    import concourse.bacc as bacc
    import concourse.bass as bass
    import concourse.tile as tile
    from concourse import mybir
    from concourse.masks import make_identity

    f32 = mybir.dt.float32
    f16 = mybir.dt.float16
    i16 = mybir.dt.int16
    i32 = mybir.dt.int32
    Alu = mybir.AluOpType
    Act = mybir.ActivationFunctionType

    nc = bacc.Bacc("TRN2", target_bir_lowering=False, debug=False,
                   num_devices=NCORES)

    xt = nc.dram_tensor("xt", [TBL_ROWS, 2 * C], f16, kind="ExternalInput")
    xc = nc.dram_tensor("xc", [128, 2, 34 * 66], f16, kind="ExternalInput")
    woff = nc.dram_tensor("woff", [128, 2, 9, 27], f16, kind="ExternalInput")
    boff = nc.dram_tensor("boff", [27, 1], f32, kind="ExternalInput")
    w2 = nc.dram_tensor("w2", [128, 9, 2, 2, 128], f16, kind="ExternalInput")
    bvec = nc.dram_tensor("bvec", [128, 2], f32, kind="ExternalInput")
    gam2 = nc.dram_tensor("gam2", [128, 2], f32, kind="ExternalInput")
    bet2 = nc.dram_tensor("bet2", [128, 2], f32, kind="ExternalInput")
    basey = nc.dram_tensor("basey", [128, PXT, 9], f32, kind="ExternalInput")
    basex = nc.dram_tensor("basex", [128, 9], f32, kind="ExternalInput")
    rep16 = nc.dram_tensor("rep16", [16, 128], f32, kind="ExternalInput")
    yout = nc.dram_tensor("yout", [CO, NPX], f16, kind="ExternalOutput")

    cc_in = nc.dram_tensor("cc_in", [1, 8], f32)
    cc_out = nc.dram_tensor("cc_out", [1, 8], f32)

    # gather source: overlapping 1024-elem windows with 512-elem stride
    xt_win = bass.AP(tensor=xt, offset=0, ap=[[2 * C, NWIN], [1, 4 * C]])

    def swap_free(ap2):
        """Swap the two free dims of a [P, A, B] AP (iteration order only)."""
        return bass.AP(tensor=ap2.tensor, offset=ap2.offset,
                       ap=[ap2.ap[0], ap2.ap[2], ap2.ap[1]])

    with tile.TileContext(nc) as tc, contextlib.ExitStack() as ctx:
        consts = ctx.enter_context(tc.tile_pool(name="consts", bufs=1))
        sb = ctx.enter_context(tc.tile_pool(name="sb", bufs=1))
        hb = ctx.enter_context(tc.tile_pool(name="hb", bufs=2))
        ps = ctx.enter_context(tc.tile_pool(name="ps", bufs=2, space="PSUM"))
        gat = ctx.enter_context(tc.tile_pool(name="gat", bufs=4))
        dpool = ctx.enter_context(tc.tile_pool(name="dpool", bufs=3))
        vals = ctx.enter_context(tc.tile_pool(name="vals", bufs=3))
        ps_y = ctx.enter_context(
            tc.tile_pool(name="ps_y", bufs=1, space="PSUM"))

        junk16 = consts.tile([128, 128], f16)
        nc.vector.memset(junk16[:], 0.5)
        # PE warm-up: absorb the cold-pipeline p-state penalty on throwaway
        # matmuls while the input DMAs are in flight.
        for i in range(30):
            pj = ps.tile([128, 512], f32, tag="ps", name="pj")
            nc.tensor.matmul(pj[:, 0:128], junk16[:], junk16[:],
                             start=True, stop=True)

        ident = consts.tile([128, 128], f32)
        make_identity(nc, ident[:])
        ident16 = consts.tile([128, 128], f16)
        nc.vector.tensor_copy(out=ident16[:], in_=ident[:])
        ones_row = consts.tile([1, 128], f32)
        nc.vector.memset(ones_row[:], 1.0)
        ones_col = consts.tile([128, 1], f32)
        nc.vector.memset(ones_col[:], 1.0)
        eps_t = consts.tile([1, 1], f32)
        nc.vector.memset(eps_t[:], GN_EPS)

        xc_sb = consts.tile([128, 2, 34 * 66], f16)
        nc.sync.dma_start(out=xc_sb[:], in_=xc[:])
        woff_sb = consts.tile([128, 2, 9, 27], f16)
        nc.sync.dma_start(out=woff_sb[:], in_=woff[:])
        boff_sb = consts.tile([27, 1], f32)
        nc.sync.dma_start(out=boff_sb[:], in_=boff[:])
        basey_sb = consts.tile([128, PXT, 9], f32)
        nc.sync.dma_start(out=basey_sb[:], in_=basey[:])
        basex_sb = consts.tile([128, 9], f32)
        nc.sync.dma_start(out=basex_sb[:], in_=basex[:])
        rep16_sb = consts.tile([16, 128], f32)
        nc.sync.dma_start(out=rep16_sb[:], in_=rep16[:])
        w2_sb = consts.tile([128, 9, 2, 2, 128], f16)
        nc.sync.dma_start(out=w2_sb[:], in_=w2[:])
        bvec_sb = consts.tile([128, 2], f32)
        nc.sync.dma_start(out=bvec_sb[:], in_=bvec[:])
        gam_sb = consts.tile([128, 2], f32)
        nc.sync.dma_start(out=gam_sb[:], in_=gam2[:])
        bet_sb = consts.tile([128, 2], f32)
        nc.sync.dma_start(out=bet_sb[:], in_=bet2[:])

        xcv = [xc_sb[:, cc, :].rearrange("p (r c) -> p r c", c=66)
               for cc in range(2)]

        # shared across halves
        wbuf = sb.tile([128, 9, 2, 8, 8], i16)     # [p, k, half, t8, u]
        nc.vector.memset(wbuf[:], 0)
        y_sb = sb.tile([128, 2, 2, 1024], f32)     # [o', oh, half, px]
        y16 = sb.tile([128, 2, 2, 1024], f16)
        s1b = sb.tile([128, 2, 2, 2], f32)     # [p, oh, half, quad]
        s2b = sb.tile([128, 2, 2, 2], f32)
        # GN bias-fold constants (b*NPX, b^2*NPX) computed early, off the tail
        bvec2n = sb.tile([128, 2], f32)
        nc.vector.tensor_scalar_mul(out=bvec2n[:], in0=bvec_sb[:],
                                    scalar1=float(NPX))
        q2n = sb.tile([128, 2], f32)
        nc.vector.tensor_tensor(out=q2n[:], in0=bvec2n[:], in1=bvec_sb[:],
                                op=Alu.mult)
        y_ps = [ps_y.tile([128, 1024], f32, tag=f"y_ps{oh}", name=f"y_ps{oh}")
                for oh in range(2)]

        # per-half state (rotating tiles, bufs=2)
        st = [{} for _ in range(2)]

        def emit_offconv(h, part, defer_bias=False):
            """part 0/1: one 512-px chunk of this half's offset conv."""
            if part == 0 and "off" not in st[h]:
                st[h]["off"] = hb.tile([27, 2, 512], f32, tag="off",
                                       name=f"off{h}")
            off_sb = st[h]["off"]
            ch = h * 2 + part
            ps_off = ps.tile([128, 512], f32, tag="psv", name="ps_off")
            st[h][f"ps_off{part}"] = ps_off
            n = 0
            for cc in range(2):
                for a in range(3):
                    for bb in range(3):
                        nc.tensor.matmul(
                            ps_off[0:27, :],
                            woff_sb[:, cc, a * 3 + bb, :],
                            xcv[cc][:, ch * 8 + a: ch * 8 + a + 8, bb: bb + 64],
                            start=(n == 0), stop=(n == 17))
                        n += 1
            if not defer_bias:
                emit_offbias(h, part)

        def emit_offbias(h, part):
            nc.vector.tensor_scalar_add(out=st[h]["off"][:, part, :],
                                        in0=st[h][f"ps_off{part}"][0:27, :],
                                        scalar1=boff_sb[:, 0:1])

        def emit_offT(h):
            offT = hb.tile([128, 8, 27], f32, tag="offT", name=f"offT{h}")
            st[h]["offT"] = offT
            off_sb = st[h]["off"]
            for t8 in range(8):
                part, sub = t8 // 4, t8 % 4
                ps_t = ps.tile([128, 27], f32, tag="ps", name="ps_t")
                nc.tensor.transpose(
                    ps_t[:, :],
                    off_sb[:, part, sub * 128:(sub + 1) * 128],
                    ident[0:27, 0:27])
                nc.scalar.copy(out=offT[:, t8, :], in_=ps_t[:, :])

        def emit_coef(h):
            offT = st[h]["offT"]
            dy = offT[:, :, 0:18:2]     # [128, 8, 9] strided views
            dx = offT[:, :, 1:18:2]
            moff = offT[:, :, 18:27]

            def ht(name):
                tl = hb.tile([128, 8, 9], f32, tag=name, name=f"{name}{h}")
                st[h][name] = tl
                return tl

            msk = ht("msk")
            nc.scalar.activation(out=msk[:], in_=moff, func=Act.Sigmoid)
            pyg = ht("pyg")
            nc.vector.tensor_tensor(out=pyg[:], in0=dy,
                                    in1=basey_sb[:, h * 8:(h + 1) * 8, :],
                                    op=Alu.add)
            pxg = ht("pxg")
            bxa = basex_sb[:]
            bx_b = bass.AP(tensor=bxa.tensor, offset=bxa.offset,
                           ap=[bxa.ap[0], [0, 8], [1, 9]])
            nc.vector.tensor_tensor(out=pxg[:], in0=dx, in1=bx_b, op=Alu.add)

            # x/y floor chains interleaved so the DVE pipelines one while the
            # other's semaphores propagate
            iiy = hb.tile([128, 8, 9], i32, tag="iiy", name="iiy")
            iix = hb.tile([128, 8, 9], i32, tag="iix", name="iix")
            nc.vector.tensor_copy(out=iiy[:], in_=pyg[:])
            nc.vector.tensor_copy(out=iix[:], in_=pxg[:])
            ffy = hb.tile([128, 8, 9], f32, tag="ffy", name="ffy")
            ffx = hb.tile([128, 8, 9], f32, tag="ffx", name="ffx")
            nc.vector.tensor_copy(out=ffy[:], in_=iiy[:])
            nc.vector.tensor_copy(out=ffx[:], in_=iix[:])
            gty = hb.tile([128, 8, 9], f32, tag="gty", name="gty")
            gtx = hb.tile([128, 8, 9], f32, tag="gtx", name="gtx")
            nc.vector.tensor_tensor(out=gty[:], in0=ffy[:], in1=pyg[:],
                                    op=Alu.is_gt)
            nc.vector.tensor_tensor(out=gtx[:], in0=ffx[:], in1=pxg[:],
                                    op=Alu.is_gt)
            y0f, ly, y0c = ht("y0f"), ht("ly"), ht("y0c")
            x0f, lx, x0c = ht("x0f"), ht("lx"), ht("x0c")
            nc.vector.tensor_tensor(out=y0f[:], in0=ffy[:], in1=gty[:],
                                    op=Alu.subtract)
            nc.vector.tensor_tensor(out=x0f[:], in0=ffx[:], in1=gtx[:],
                                    op=Alu.subtract)
            nc.vector.tensor_tensor(out=ly[:], in0=pyg[:], in1=y0f[:],
                                    op=Alu.subtract)
            nc.vector.tensor_tensor(out=lx[:], in0=pxg[:], in1=x0f[:],
                                    op=Alu.subtract)
            nc.vector.tensor_scalar(out=y0c[:], in0=y0f[:], scalar1=0.0,
                                    scalar2=65.0, op0=Alu.max, op1=Alu.min)
            nc.vector.tensor_scalar(out=x0c[:], in0=x0f[:], scalar1=0.0,
                                    scalar2=65.0, op0=Alu.max, op1=Alu.min)
            # clamping maps x0<=-1 (y0<=-1) onto column/row 0: the +1-corner
            # slot then reads wrong data; kill +1 coefs (true values are 0)
            # by folding the masks into lx (x side) and ay1 (y side).
            mxv, myv = ht("mxv"), ht("myv")
            nc.vector.tensor_scalar(out=myv[:], in0=y0f[:], scalar1=0.0,
                                    scalar2=None, op0=Alu.is_ge)
            nc.vector.tensor_scalar(out=mxv[:], in0=x0f[:], scalar1=0.0,
                                    scalar2=None, op0=Alu.is_ge)
            ly1, lx1 = ht("ly1"), ht("lx1")
            nc.scalar.activation(out=ly1[:], in_=ly[:], func=Act.Copy,
                                 scale=-1.0, bias=1.0)
            nc.scalar.activation(out=lx1[:], in_=lx[:], func=Act.Copy,
                                 scale=-1.0, bias=1.0)
            lxm = ht("lxm")
            ay0, ay1 = ht("ay0"), ht("ay1")
            for n in ("c00", "c01", "c10", "c11"):
                ht(n)

        def emit_cmul(h):
            msk, ly, ly1, lx1 = (st[h][n] for n in ("msk", "ly", "ly1", "lx1"))
            lx, mxv, myv, lxm = (st[h][n] for n in ("lx", "mxv", "myv", "lxm"))
            ay0, ay1 = st[h]["ay0"], st[h]["ay1"]
            c00, c01 = st[h]["c00"], st[h]["c01"]
            c10, c11 = st[h]["c10"], st[h]["c11"]
            nc.vector.tensor_tensor(out=lxm[:], in0=lx[:], in1=mxv[:],
                                    op=Alu.mult)
            nc.vector.tensor_tensor(out=ay0[:], in0=ly1[:], in1=msk[:],
                                    op=Alu.mult)
            nc.vector.tensor_tensor(out=ay1[:], in0=ly[:], in1=msk[:],
                                    op=Alu.mult)
            nc.vector.tensor_tensor(out=ay1[:], in0=ay1[:], in1=myv[:],
                                    op=Alu.mult)
            nc.vector.tensor_tensor(out=c00[:], in0=ay0[:], in1=lx1[:],
                                    op=Alu.mult)
            nc.vector.tensor_tensor(out=c01[:], in0=ay0[:], in1=lxm[:],
                                    op=Alu.mult)
            nc.vector.tensor_tensor(out=c10[:], in0=ay1[:], in1=lx1[:],
                                    op=Alu.mult)
            nc.vector.tensor_tensor(out=c11[:], in0=ay1[:], in1=lxm[:],
                                    op=Alu.mult)

        def emit_idx(h):
            # idxf [128, 9k, 8t] fp32 ; f = k*8 + t ; idx = y0c*66 + x0c
            idxf = hb.tile([128, 9, 8], f32, tag="idxf", name=f"idxf{h}")
            tmpi = hb.tile([128, 8, 9], f32, tag="tmpi", name="tmpi")
            nc.vector.tensor_scalar_mul(out=tmpi[:], in0=st[h]["y0c"][:],
                                        scalar1=66.0)
            nc.vector.tensor_tensor(out=swap_free(idxf[:]), in0=tmpi[:],
                                    in1=st[h]["x0c"][:], op=Alu.add)
            # transpose [128p, 72f] -> [72, 128] -> wrap into 16-partition
            # groups replicated x8 (each Q7 core reads its own group)
            idxf_flat = idxf[:].rearrange("p a b -> p (a b)")
            ps_t1 = ps.tile([72, 128], f32, tag="ps", name="ps_t1")
            nc.tensor.transpose(ps_t1[:, :], idxf_flat[:, :], ident[:, :])
            t1sb = hb.tile([72, 128], f32, tag="t1sb", name="t1sb")
            nc.scalar.copy(out=t1sb[:], in_=ps_t1[:, :])
            for u in range(8):
                ps_t2 = ps.tile([16, 72], f32, tag="ps", name="ps_t2")
                nc.tensor.transpose(ps_t2[:, :], t1sb[:, u * 16:(u + 1) * 16],
                                    ident[0:72, 0:72])
                t2sb = hb.tile([16, 72], f32, tag="t2sb", name="t2sb")
                nc.scalar.copy(out=t2sb[:], in_=ps_t2[:, :])
                ps_rep = ps.tile([128, 72], f32, tag="ps", name="ps_rep")
                nc.tensor.matmul(ps_rep[:, :], rep16_sb[:, :], t2sb[:, :],
                                 start=True, stop=True)
                nc.vector.tensor_copy(
                    out=wbuf[:, :, h, :, u],
                    in_=ps_rep[:, :].rearrange("p (a t) -> p a t", t=8))

        def emit_k(h, k):
            g = gat.tile([128, 8, 1024], f16, tag="g", name="g")
            nc.gpsimd.dma_gather(
                out_ap=g[:], in_ap=xt_win,
                idxs_ap=wbuf[:, k, h, :, :],
                num_idxs=1024, num_idxs_reg=1024,
                elem_size=1024, elem_step=512, queue_num=0)
            coefs = (st[h]["c00"], st[h]["c10"], st[h]["c01"], st[h]["c11"])
            dk = dpool.tile([128, 8, 4, 128], f16, tag="dk", name="dk")
            for t8 in range(8):
                for ci in range(4):
                    nc.vector.tensor_scalar_mul(
                        out=dk[:, t8, ci, :], in0=ident16[:],
                        scalar1=coefs[ci][:, t8, k:k + 1])
            valTs = []
            for quad in range(2):
                valT = vals.tile([128, 1024], f16, tag="valT", name="valT")
                valTs.append(valT)
                for pair in range(2):
                    ps_v = ps.tile([128, 512], f32, tag="psv", name="ps_v")
                    for sub in range(2):
                        t8 = quad * 4 + pair * 2 + sub
                        for ci in range(4):
                            for cc in range(2):
                                nc.tensor.matmul(
                                    ps_v[:, sub * 256 + cc * 128:
                                         sub * 256 + (cc + 1) * 128],
                                    g[:, t8, ci * 256 + cc * 128:
                                      ci * 256 + (cc + 1) * 128],
                                    dk[:, t8, ci, :],
                                    start=(sub == 0 and ci == 0 and cc == 0),
                                    stop=(sub == 1 and ci == 3 and cc == 1))
                    nc.scalar.copy(out=valT[:, pair * 512:(pair + 1) * 512],
                                   in_=ps_v[:])
            for quad in range(2):
                vv = valTs[quad][:].rearrange("p (s c x) -> p s c x",
                                              c=2, x=128)
                for oh in range(2):
                    for cc in range(2):
                        # one 512-px matmul per PSUM bank (oh, quad)
                        nc.tensor.matmul(
                            y_ps[oh][:, quad * 512:(quad + 1) * 512],
                            w2_sb[:, k, cc, oh, :],
                            vv[:, :, cc, :],
                            start=(k == 0 and cc == 0),
                            stop=(k == KK - 1 and cc == 1))

        def emit_yevict(h):
            for quad in range(2):
                for oh in range(2):
                    sl = slice(quad * 512, (quad + 1) * 512)
                    sq_scratch = sb.tile([128, 512], f32, tag="sq", name="sq")
                    nc.scalar.activation(out=y_sb[:, oh, h, sl],
                                         in_=y_ps[oh][:, sl],
                                         func=Act.Copy,
                                         accum_out=s1b[:, oh, h, quad:quad+1])
                    nc.scalar.activation(out=sq_scratch[:],
                                         in_=y_ps[oh][:, sl],
                                         func=Act.Square,
                                         accum_out=s2b[:, oh, h, quad:quad+1])

        # ------------- emission schedule (PE executes in program order) ----
        emit_offconv(0, 0)
        emit_offconv(0, 1)
        emit_offT(0)
        emit_coef(0)
        emit_offconv(1, 0, defer_bias=True)
        emit_offconv(1, 1, defer_bias=True)
        emit_idx(0)
        emit_offbias(1, 0)
        emit_offbias(1, 1)
        emit_cmul(0)
        emit_k(0, 0)
        emit_offT(1)
        emit_k(0, 1)
        emit_coef(1)
        emit_k(0, 2)
        emit_idx(1)
        emit_k(0, 3)
        emit_cmul(1)
        for k in range(4, KK):
            emit_k(0, k)
        emit_yevict(0)
        for k in range(KK):
            emit_k(1, k)
        emit_yevict(1)

        # ---------------- GroupNorm tail -----------------------------------
        s1 = sb.tile([128, 2], f32)
        s2 = sb.tile([128, 2], f32)
        s1h = sb.tile([128, 2, 2], f32)
        s2h = sb.tile([128, 2, 2], f32)
        nc.vector.tensor_tensor(out=s1h[:], in0=s1b[:, :, :, 0],
                                in1=s1b[:, :, :, 1], op=Alu.add)
        nc.vector.tensor_tensor(out=s2h[:], in0=s2b[:, :, :, 0],
                                in1=s2b[:, :, :, 1], op=Alu.add)
        nc.vector.tensor_tensor(out=s1[:], in0=s1h[:, :, 0], in1=s1h[:, :, 1],
                                op=Alu.add)
        nc.vector.tensor_tensor(out=s2[:], in0=s2h[:, :, 0], in1=s2h[:, :, 1],
                                op=Alu.add)
        # fold conv bias b: S1' = S1 + NPX*b ; S2' = S2 + 2 b S1 + NPX b^2
        stk = sb.tile([128, 4], f32)
        q1 = sb.tile([128, 2], f32)
        nc.vector.tensor_tensor(out=q1[:], in0=bvec_sb[:], in1=s1[:],
                                op=Alu.mult)
        nc.vector.scalar_tensor_tensor(out=stk[:, 2:4], in0=q1[:], scalar=2.0,
                                       in1=s2[:], op0=Alu.mult, op1=Alu.add)
        nc.vector.tensor_tensor(out=stk[:, 2:4], in0=q2n[:], in1=stk[:, 2:4],
                                op=Alu.add)
        nc.vector.tensor_tensor(out=stk[:, 0:2], in0=bvec2n[:], in1=s1[:],
                                op=Alu.add)
        ps_s = ps.tile([1, 4], f32, tag="ps", name="ps_s")
        nc.tensor.matmul(ps_s[:, :], ones_col[:, :], stk[:, :],
                         start=True, stop=True)
        tot4 = sb.tile([1, 4], f32)
        nc.vector.tensor_copy(out=tot4[:], in_=ps_s[:, :])
        ccs = sb.tile([1, 8], f32)
        nc.vector.memset(ccs[:], 0.0)
        nc.vector.tensor_tensor(out=ccs[:, 0:1], in0=tot4[:, 0:1],
                                in1=tot4[:, 1:2], op=Alu.add)
        nc.vector.tensor_tensor(out=ccs[:, 1:2], in0=tot4[:, 2:3],
                                in1=tot4[:, 3:4], op=Alu.add)

        tot = sb.tile([1, 8], f32)
        if use_collective:
            nc.sync.dma_start(out=cc_in[:], in_=ccs[:])
            nc.gpsimd.collective_compute(
                "AllReduce", Alu.add,
                replica_groups=[[0, 1], [2, 3], [4, 5], [6, 7]],
                ins=[cc_in[:].opt()], outs=[cc_out[:].opt()])
            nc.sync.dma_start(out=tot[:], in_=cc_out[:])
        else:
            nc.vector.tensor_scalar_mul(out=tot[:], in0=ccs[:], scalar1=2.0)

        invN = 1.0 / float(C * H * W)
        mu = sb.tile([1, 1], f32)
        nc.vector.tensor_scalar_mul(out=mu[:], in0=tot[:, 0:1], scalar1=invN)
        mu2 = sb.tile([1, 1], f32)
        nc.vector.tensor_tensor(out=mu2[:], in0=mu[:], in1=mu[:], op=Alu.mult)
        var = sb.tile([1, 1], f32)
        nc.vector.scalar_tensor_tensor(out=var[:], in0=tot[:, 1:2],
                                       scalar=invN, in1=mu2[:],
                                       op0=Alu.mult, op1=Alu.subtract)
        std = sb.tile([1, 1], f32)
        nc.scalar.activation(out=std[:], in_=var[:], func=Act.Sqrt,
                             bias=eps_t[:, 0:1])
        rs = sb.tile([1, 1], f32)
        nc.vector.reciprocal(out=rs[:], in_=std[:])
        mr = sb.tile([1, 2], f32)
        nc.vector.tensor_copy(out=mr[:, 0:1], in_=mu[:])
        nc.vector.tensor_copy(out=mr[:, 1:2], in_=rs[:])
        ps_b = ps.tile([128, 2], f32, tag="ps", name="ps_b")
        nc.tensor.matmul(ps_b[:, :], ones_row[:, :], mr[:, :],
                         start=True, stop=True)
        mr128 = sb.tile([128, 2], f32)
        nc.vector.tensor_copy(out=mr128[:], in_=ps_b[:, :])
        svec = sb.tile([128, 2], f32)
        nc.vector.tensor_scalar_mul(out=svec[:], in0=gam_sb[:],
                                    scalar1=mr128[:, 1:2])
        tdiff = sb.tile([128, 2], f32)
        nc.vector.tensor_scalar_sub(out=tdiff[:], in0=bvec_sb[:],
                                    scalar1=mr128[:, 0:1])
        b2 = sb.tile([128, 2], f32)
        nc.vector.tensor_tensor(out=b2[:], in0=tdiff[:], in1=svec[:],
                                op=Alu.mult)
        nc.vector.tensor_tensor(out=b2[:], in0=b2[:], in1=bet_sb[:],
                                op=Alu.add)

        for oh in range(2):
            for half in range(2):
                nc.scalar.activation(out=y16[:, oh, half, :],
                                     in_=y_sb[:, oh, half, :],
                                     func=Act.Relu,
                                     scale=svec[:, oh:oh + 1],
                                     bias=b2[:, oh:oh + 1])
                nc.sync.dma_start(
                    out=yout[oh * 128:(oh + 1) * 128,
                             half * 1024:(half + 1) * 1024],
                    in_=y16[:, oh, half, :])

    nc.compile()
    return nc


# ----------------------------------------------------------------- entry
def kernel(x, w_off, b_off, w, b, gamma, beta):
    from concourse.bass_utils import run_bass_kernel_spmd

    in_maps = prep_per_core(np.asarray(x, np.float32),
                            np.asarray(w_off, np.float32),
                            np.asarray(b_off, np.float32),
                            np.asarray(w, np.float32),
                            np.asarray(b, np.float32),
                            np.asarray(gamma, np.float32),
                            np.asarray(beta, np.float32))
    if "nc" not in _cache:
        _cache["nc"] = build_module(use_collective=True)
    res = run_bass_kernel_spmd(_cache["nc"], in_maps,
                               core_ids=list(range(NCORES)))
    out = np.zeros((B, CO, H, W), np.float32)
    for core in range(NCORES):
        bi, hh = core // 2, core % 2
        out[bi, :, hh * 32:(hh + 1) * 32, :] = (
            res.results[core]["yout"].reshape(CO, 32, 64))
    return out
